# revision 66
# baseline (speedup 1.0000x reference)
"""Trainium2 Bass kernel for a dense multi-head attention layer (v5).

Problem shapes (hardcoded): B=2, S=4096, H=512, NH=8, HD=64.
Sharding: 16 (batch, head) pairs -> 2 heads per core across 8 cores.

v5 (~275-278us) vs v4: phase A rebalanced (V-copies on ScalarE since
VectorE was overcommitted there; s2p bufs=3 with 3-ktile groups); the
DVE exp takes the FIRST tile of each phase-B group (its scores finish
~230ns earlier, and the 1.4us DVE op then fits the sc-slot reuse
window where the 1.33us SE exp fits the later slot); a new chunk's
first PV defers one group while the previous chunk's evac chain
drains; oproj output copies alternate SE/VE.

v4 changes vs v2 (301.3us -> ~281.5us):
- K-tile groups of 2 batch the PE array-tiling mode switches (row-tiled
  64x128 concurrent scores pairs vs full-array 128x65 PV): each switch
  drains the PE (~100ns), so batching halves that cost.
- O-projection interleaved into the main loop (was a 34us tail running
  at half clock after the HAM saw >3us of PE idle), fused to ONE
  matmul per token tile: ctx rows are pre-scaled by 1/softmax-sum at
  evacuation (SE row copy -> VE reciprocal_approx_fast -> GpSimd
  partition_broadcast -> VE scalar_tensor_tensor), so both heads'
  64-dim blocks concatenate into one K=128 contraction with Wo.
- Per-chunk ctxT tiles avoid false whole-tile deps (evac writes vs
  oproj reads of other chunks).
- Chunk-0 PV is deferred to phase B (pb pool holds the whole chunk) so
  phase A PSUM fits pj+s2p and phase B gets cx triple-buffering, which
  removes chunk-boundary PE stalls (HAM downclock triggers).
- Softmax exp split ScalarE (ACT Exp) / VectorE (custom DVE op
  EXP2R_ANT, bf16 bits via int16, scores pre-scaled on host).

HW facts this leans on (measured): VE ops mis-read PSUM rows at
partition offsets (hop through SE copy first); GpSimd cannot touch
PSUM and its bulk elementwise is ~20x slower than VE (only
partition_broadcast is cheap); DMA cannot read PSUM.
"""

import numpy as np

B, S, H, NH, HD = 2, 4096, 512, 8, 64
N_CORES = 8

_CACHE = {}

# ---- custom DVE op: 2^(Y/128) -> bf16 bits via int16 ----------------------
EXP2_NAME = "EXP2R_ANT"
KM_VAL = float(3 * 2**29)          # magic: round to multiple of 128
EXP2_A = -0.0026979539543390274    # quadratic coeff (F^2)
EXP2_B = -0.0020175932440906763      # quadratic coeff (F)
EXP2_CE = 16253.2333984375           # 128*127 + fit const (+trunc offset)
LN2_128 = float(np.log(2.0) / 128.0)
SCALE_Q = float(128.0 * np.log2(np.e) / np.sqrt(HD))


def _exp2_reference(in0, in1, s0, s1, imm2):
    f32 = np.float32
    KM = f32(KM_VAL)
    Y = in0.astype(f32)
    G = ((Y + KM).astype(f32) - KM).astype(f32)
    F = (Y - G).astype(f32)
    return ((Y + f32(s0)).astype(f32)
            + (((F * f32(s1)).astype(f32) + f32(imm2)).astype(f32)
               * F).astype(f32)).astype(f32)


def _get_exp2_op():
    import concourse.dve_ops as dvo
    for op in dvo.OPS:
        if op.name == EXP2_NAME:
            return op
    from concourse.dve_spec import (
        C0, C1, C2, C3, Spec, Src0, _spill_c3_to_src1, lower,
    )
    from concourse.dve_uop import DveOpSpec

    Y = Src0
    G = (Y + C3) - C3
    F = Y - G
    body = (Y + C0) + ((F * C1 + C2) * F)
    body = _spill_c3_to_src1(body)
    spec = Spec(body=body, reference=_exp2_reference)

    row = dvo._CUSTOM_DVE_ROW_BASE + len(dvo.OPS)
    shas = {}
    for ver in ("v3", "v4"):
        try:
            uops = lower(spec, ver=ver)
            shas[ver] = DveOpSpec(
                name=EXP2_NAME, opcode=row, uops=uops, rd1_en=True
            ).sha(ver)
        except Exception:
            pass
    assert shas, "EXP2R_ANT failed to lower"
    op = dvo.DveOp(EXP2_NAME, spec, subdim=False, uops_sha=shas)
    dvo.OPS.append(op)
    dvo._SUB_OPCODE_FOR_NAME[EXP2_NAME] = row
    dvo.CUSTOM_DVE_SPECS[EXP2_NAME] = spec
    return op


def _emit(nc, tc, ctx, aps, S_):
    from collections import deque

    import concourse.mybir as mybir

    f32 = mybir.dt.float32
    bf16 = mybir.dt.bfloat16
    i16 = mybir.dt.int16
    Exp = mybir.ActivationFunctionType.Exp
    mult = mybir.AluOpType.mult

    exp2_op = _get_exp2_op()

    NQ = S_ // 512   # query chunks
    NK = S_ // 128   # key tiles
    NS = S_ // 128   # output row tiles

    P = ctx.enter_context(tc.tile_pool(name="persist", bufs=1))

    wk_sb = P.tile([128, 512], bf16, tag="wk")
    nc.sync.dma_start(wk_sb[:], aps["wk"])
    bk_sb = P.tile([128, 1], f32, tag="bk")
    wq_sb = P.tile([128, 512], bf16, tag="wq")
    bq_sb = P.tile([128, 1], f32, tag="bq")
    wv_sb = P.tile([128, 512], bf16, tag="wv")
    wo_sb = P.tile([128, 512], bf16, tag="wo")

    qT = P.tile([128, S_], bf16, tag="qT")         # q^T, 2 heads on partitions
    kT = P.tile([128, S_], bf16, tag="kT")
    Vt = P.tile([128, NK * 130], bf16, tag="V")    # per k-tile: [h0|1s|h1|1s]
    # scaled ctx, one tile per chunk (separate tiles avoid false whole-tile
    # deps between each chunk's evac write and other chunks' oproj reads)
    ctxTs = [P.tile([128, 512], bf16, tag=f"ctxT{c}", name=f"ctxT{c}")
             for c in range(NQ)]
    km = P.tile([128, 1], f32, tag="km")
    nc.vector.memset(km[:], KM_VAL)
    ones1 = P.tile([128, 1], f32, tag="ones1")
    nc.vector.memset(ones1[:], 1.0)
    vt_cols = Vt.rearrange("p (n c) -> p n c", c=65)
    nc.vector.tensor_copy(
        vt_cols[:, :, 64:65], ones1[:, 0:1].to_broadcast((128, 2 * NK, 1))
    )

    nmp = ctx.enter_context(tc.tile_pool(name="nm", bufs=2))
    # Holds all of chunk 0's probs (PV for chunk 0 is deferred to phase B so
    # phase A PSUM fits in pj+s2p) plus the steady-state pipeline depth.
    pbp = ctx.enter_context(tc.tile_pool(name="pb", bufs=34))

    def scores_mm(sc_slice, h, t, c):
        nc.tensor.matmul(
            sc_slice,
            kT[h * 64:(h + 1) * 64, t * 128:(t + 1) * 128],
            qT[h * 64:(h + 1) * 64, c * 512:(c + 1) * 512],
            start=True, stop=True,
        )

    def pv_mm(cxt, h, t, pb_slice):
        nc.tensor.matmul(
            cxt,
            Vt[:, t * 130 + h * 65:t * 130 + (h + 1) * 65],
            pb_slice,
            start=(t == 0), stop=(t == NK - 1),
        )

    def do_exp(sc, pb, use_dve):
        # sc: [128, 1024] f32 PSUM (scores pre-scaled to Y = 128*log2e*s/8)
        # pb: [128, 1024] int16 SBUF; bf16 bit patterns of exp(s/8) land here.
        if use_dve:
            nc.vector._custom_dve(
                exp2_op, out=pb[:], in0=sc[:], in1=km[:, 0:1],
                s0=EXP2_CE, s1=EXP2_A, imm2=EXP2_B,
            )
        else:
            nc.scalar.activation(pb.bitcast(bf16), sc[:], Exp, scale=LN2_128)

    # ---- evacuation: reciprocal of sums row, broadcast, scale ctx --------
    rec_bs = {}

    def make_steps(cx, c2):
        # Paced steps per chunk. s2a (ScalarE) evacuates the cx PSUM slot as
        # soon as possible; the 1/sums chain (SE copy -> VE recip -> gpsimd
        # bcast) runs in parallel; the final multiply runs on idle GpSimd so
        # no step waits behind the VE/SE exp backlog.
        def s1(h):
            def f():
                # VE ops mis-read PSUM rows at partition offsets on HW; hop
                # the sums row to SBUF partition 0 via ScalarE first.
                srow = nmp.tile([1, 512], f32, tag=f"srow{h}", name=f"srow{h}")
                nc.scalar.copy(srow[:], cx[h][64:65, :])
                rec = nmp.tile([1, 512], f32, tag=f"rec{h}", name=f"rec{h}")
                nc.vector.reciprocal_approx_fast(rec[:], srow[:])
                rb = nmp.tile([64, 512], f32, tag=f"recb{h}", name=f"recb{h}")
                nc.gpsimd.partition_broadcast(rb[:], rec[0:1, :])
                rec_bs[(c2, h)] = rb
            return f

        def s2(h):
            def f():
                rb = rec_bs.pop((c2, h))
                nc.vector.scalar_tensor_tensor(
                    ctxTs[c2][h * 64:(h + 1) * 64, :],
                    cx[h][0:64, :], 1.0, rb[:],
                    op0=mult, op1=mult,
                )
            return f
        return [s1(0), s1(1), s2(0), s2(1)]

    obp = ctx.enter_context(tc.tile_pool(name="ob", bufs=3))

    def oproj_tile(st, opp):
        ps = opp.tile([128, 512], f32, tag="op", name="ps")
        nc.tensor.matmul(
            ps[:],
            ctxTs[st // 4][:, (st % 4) * 128:(st % 4 + 1) * 128],
            wo_sb[:],
            start=True, stop=True,
        )
        ob = obp.tile([128, 512], bf16, tag="ob", name="ob")
        if st % 2 == 0:
            nc.scalar.copy(ob[:], ps[:])
        else:
            nc.vector.tensor_copy(ob[:], ps[:])
        nc.sync.dma_start(aps["outp"][st * 128:(st + 1) * 128, :], ob[:])

    # ---- Phase A: projections interleaved with chunk-0 attention ---------
    pend = deque()
    steps = deque()  # ordered (kind, fn): evac s1/s2 then that chunk's oproj
    cxs = {}
    opp_holder = []

    def pop_steps(n):
        popped_op = False
        for _ in range(n):
            if not steps:
                break
            kind, fn = steps[0]
            if kind == "op" and popped_op:
                break
            steps.popleft()
            popped_op |= kind == "op"
            if fn is not None:
                fn()

    with tc.tile_pool(name="xt", bufs=1) as xtp, \
         tc.tile_pool(name="pj", bufs=2, space="PSUM") as pj, \
         tc.tile_pool(name="s2", bufs=3, space="PSUM") as s2p:
        xts = []
        for j in range(4):
            t = xtp.tile([128, S_], bf16, tag=f"xt{j}")
            xts.append(t)
        for j in range(4):
            nc.sync.dma_start(
                xts[j][:, 0:512], aps["xT"][j * 128:(j + 1) * 128, 0:512]
            )
        nc.sync.dma_start(wq_sb[:], aps["wq"])
        nc.sync.dma_start(bk_sb[:], aps["bk"])
        nc.sync.dma_start(bq_sb[:], aps["bq"])
        nc.sync.dma_start(wv_sb[:], aps["wv"])
        for c in range(1, NQ):
            for j in range(4):
                nc.sync.dma_start(
                    xts[j][:, c * 512:(c + 1) * 512],
                    aps["xT"][j * 128:(j + 1) * 128, c * 512:(c + 1) * 512],
                )
        nc.sync.dma_start(wo_sb[:], aps["wo"])

        def kq_proj_chunk(wsb, bsb, dst, c):
            ps = pj.tile([128, 512], f32, tag="pj", name="pjt")
            for j in range(4):
                nc.tensor.matmul(
                    ps[:],
                    wsb[:, j * 128:(j + 1) * 128],
                    xts[j][:, c * 512:(c + 1) * 512],
                    start=(j == 0), stop=(j == 3),
                )
            nc.vector.tensor_scalar_add(
                dst[:, c * 512:(c + 1) * 512], ps[:], bsb[:, 0:1]
            )

        def v_proj_tile(t):
            ps = pj.tile([128, 128], f32, tag="pj", name="pjv")
            for j in range(4):
                nc.tensor.matmul(
                    ps[:],
                    xts[j][:, t * 128:(t + 1) * 128],
                    wv_sb[:, j * 128:(j + 1) * 128],
                    start=(j == 0), stop=(j == 3),
                )
            # ScalarE: phase A's VectorE is the overcommitted engine (exps +
            # bias adds); SE has slack here.
            dst = Vt[:, t * 130:t * 130 + 130].rearrange(
                "p (two c) -> p two c", c=65)[:, :, 0:64]
            src = ps.rearrange("p (two c) -> p two c", c=64)
            nc.scalar.copy(dst, src)

        kq_proj_chunk(wk_sb, bk_sb, kT, 0)
        kq_proj_chunk(wq_sb, bq_sb, qT, 0)
        NVUP = min(1, NK)
        for t in range(NVUP):
            v_proj_tile(t)

        # Interleave: 3 V tiles then one K chunk, repeating, so V stays just
        # ahead of chunk-0 PV consumption and K chunk c lands before tile 4c.
        tasks = []
        vq = list(range(NVUP, NK))
        kq = list(range(1, NQ))
        while vq or kq:
            for _ in range(3):
                if vq:
                    tasks.append(("v", vq.pop(0)))
            if kq:
                tasks.append(("k", kq.pop(0)))
        tasks += [("q", c) for c in range(1, NQ)]
        ti = 0

        GA = 3  # phase A group size (= s2p bufs; one group of exp slack)
        for gstart in range(0, NK, GA):
            for t in range(gstart, min(gstart + GA, NK)):
                sc = s2p.tile([128, 1024], f32, tag="s2", name="s2")
                scores_mm(sc[:, 0:512], 0, t, 0)
                scores_mm(sc[:, 512:1024], 1, t, 0)
                pb = pbp.tile([128, 1024], i16, tag="pb", name="pb")
                do_exp(sc, pb, use_dve=(t % 2 == 0))
                pend.append((0, t, pb.bitcast(bf16)))
            # full-array phase: projections + delayed PV
            t_hi = min(gstart + GA, NK) - 1
            while ti * NK < (t_hi + 1) * len(tasks):
                kind, v = tasks[ti]
                ti += 1
                if kind == "k":
                    kq_proj_chunk(wk_sb, bk_sb, kT, v)
                elif kind == "q":
                    kq_proj_chunk(wq_sb, bq_sb, qT, v)
                else:
                    v_proj_tile(v)

    # ---- Phase B: chunks 1..NQ-1, grouped mode-batched pipeline ----------
    with tc.tile_pool(name="sc", bufs=2, space="PSUM") as scp, \
         tc.tile_pool(name="op", bufs=1, space="PSUM") as opp, \
         tc.tile_pool(name="cx", bufs=3, space="PSUM") as cxp:
        opp_holder.append(opp)
        deferred = set()

        def flush_one():
            c2, t2, pbb = pend.popleft()
            if t2 == 0:
                cxs[c2] = {h: cxp.tile([65, 512], f32, tag="cx",
                                       name=f"cx{h}") for h in (0, 1)}
            cx = cxs[c2]
            pv_mm(cx[0][:], 0, t2, pbb[:, 0:512])
            pv_mm(cx[1][:], 1, t2, pbb[:, 512:1024])
            if t2 == NK - 1:
                for s in make_steps(cx, c2):
                    steps.append(("e", s))
                # pacing slots: give the VE evac chain ~2 groups of headroom
                # before the first oproj matmul enters the PE FIFO
                steps.append(("e", None))
                steps.append(("e", None))
                for j in range(4):
                    steps.append(
                        ("op",
                         lambda st=4 * c2 + j: oproj_tile(st, opp_holder[0])))
                del cxs[c2]

        for c in range(1, NQ):
            for g in range(NK // 2):
                # scores first (tiled mode, back-to-back pairs)
                scg = []
                for t in (2 * g, 2 * g + 1):
                    sc = scp.tile([128, 1024], f32, tag="sc", name="sc")
                    scores_mm(sc[:, 0:512], 0, t, c)
                    scores_mm(sc[:, 512:1024], 1, t, c)
                    scg.append((t, sc))
                # paced evac/oproj steps: their SE/VE ops enqueue ahead of
                # this group's exps so the chain isn't stuck behind them.
                pop_steps(3)
                for t, sc in scg:
                    pb = pbp.tile([128, 1024], i16, tag="pb", name="pb")
                    do_exp(sc, pb, use_dve=(t % 2 == 0))
                    pend.append((c, t, pb.bitcast(bf16)))
                thr = 6
                nfl = 0
                while len(pend) > thr and nfl < 3:
                    # defer a new chunk's first PV by one group while the
                    # previous chunk's evac chain is still draining: ~0.9us
                    # of PE idle now beats ~1.4us of cx-slot wait next group
                    if pend[0][1] == 0 and steps and c not in deferred:
                        deferred.add(c)
                        # keep the PE "busy" through the deferral bubble so
                        # the HAM clock-gate doesn't halve the clock: dummy
                        # weight loads (every real matmul reloads its own
                        # weights, so the background buffer is scratch)
                        for _ in range(8):
                            nc.tensor.ldweights(wk_sb[:, 0:128])
                        break
                    flush_one()
                    nfl += 1
        while pend:
            flush_one()
        # drain the last chunk's evacuation inside these pools; its oproj
        # tiles run after, in a wider PSUM pool so they overlap
        while steps and steps[0][0] == "e":
            _, fn = steps.popleft()
            if fn is not None:
                fn()

    with tc.tile_pool(name="op4", bufs=4, space="PSUM") as op4:
        opp_holder[0] = op4
        # keep the PE clock at 8/8 through the final evac chain so the
        # trailing oproj matmuls don't run at half clock
        for _ in range(30):
            nc.tensor.ldweights(wk_sb[:, 0:128])
        while steps:
            _, fn = steps.popleft()
            if fn is not None:
                fn()


def _build(S_=S):
    from contextlib import ExitStack

    import concourse.mybir as mybir
    import concourse.tile as tile
    from concourse import bacc

    f32 = mybir.dt.float32
    bf16 = mybir.dt.bfloat16
    nc = bacc.Bacc("TRN2", target_bir_lowering=False, debug=False,
                   num_devices=N_CORES)
    aps = {
        "xT": nc.dram_tensor("xT", [H, S_], bf16, kind="ExternalInput").ap(),
        "wq": nc.dram_tensor("wq", [128, H], bf16, kind="ExternalInput").ap(),
        "wk": nc.dram_tensor("wk", [128, H], bf16, kind="ExternalInput").ap(),
        "wv": nc.dram_tensor("wv", [128, H], bf16, kind="ExternalInput").ap(),
        "wo": nc.dram_tensor("wo", [128, H], bf16, kind="ExternalInput").ap(),
        "bq": nc.dram_tensor("bq", [128, 1], f32, kind="ExternalInput").ap(),
        "bk": nc.dram_tensor("bk", [128, 1], f32, kind="ExternalInput").ap(),
        "outp": nc.dram_tensor("outp", [S_, H], bf16, kind="ExternalOutput").ap(),
    }
    with tile.TileContext(nc) as tc:
        with ExitStack() as stack:
            _emit(nc, tc, stack, aps, S_)
    nc.compile()
    return nc


def _rearrange_w(wT_slice):
    # [512, 128] (h_in, d) -> [128, 512] where col t*128+d holds wT[t*128+p, d]
    import ml_dtypes

    return np.ascontiguousarray(
        wT_slice.reshape(4, 128, 128).transpose(1, 0, 2).reshape(128, 512)
    ).astype(ml_dtypes.bfloat16)


def _host_prep(hidden_states, Wq, bq, Wk, bk, Wv, bv, Wo, bo, S_=S):
    import ml_dtypes

    bf = ml_dtypes.bfloat16
    xT = [np.ascontiguousarray(hidden_states[b].T).astype(bf)
          for b in range(B)]
    in_maps = []
    for c in range(N_CORES):
        b, p = c // 4, c % 4
        sl = slice(p * 128, (p + 1) * 128)
        in_maps.append({
            "xT": xT[b],
            "wq": _rearrange_w(Wq.T[:, sl].astype(np.float32) * SCALE_Q),
            "wk": _rearrange_w(Wk.T[:, sl].astype(np.float32)),
            "wv": _rearrange_w(Wv.T[:, sl].astype(np.float32)),
            "wo": np.ascontiguousarray(
                Wo.T[p * 128:(p + 1) * 128, :]).astype(bf),
            "bq": np.ascontiguousarray(
                (bq[sl].astype(np.float32) * SCALE_Q).reshape(128, 1)),
            "bk": np.ascontiguousarray(
                bk[sl].astype(np.float32).reshape(128, 1)),
        })
    return in_maps


def kernel(hidden_states, Wq, bq, Wk, bk, Wv, bv, Wo, bo):
    from concourse.bass_utils import run_bass_kernel_spmd

    hidden_states = np.asarray(hidden_states)
    Wq, bq = np.asarray(Wq), np.asarray(bq)
    Wk, bk = np.asarray(Wk), np.asarray(bk)
    Wv, bv = np.asarray(Wv), np.asarray(bv)
    Wo, bo = np.asarray(Wo), np.asarray(bo)

    if "nc" not in _CACHE:
        _CACHE["nc"] = _build(S)
    nc = _CACHE["nc"]

    in_maps = _host_prep(hidden_states, Wq, bq, Wk, bk, Wv, bv, Wo, bo)
    res = run_bass_kernel_spmd(nc, in_maps, core_ids=list(range(N_CORES)))

    # bv contributes bv @ Wo.T (softmax weights sum to 1); bo added once.
    bo_eff = (bo.astype(np.float64) +
              bv.astype(np.float64) @ Wo.T.astype(np.float64)).astype(np.float32)
    out = np.empty((B, S, H), dtype=np.float32)
    for b in range(B):
        acc = np.zeros((S, H), dtype=np.float32)
        for p in range(4):
            acc += res.results[b * 4 + p]["outp"].astype(np.float32)
        out[b] = acc + bo_eff
    return out


# revision 67
# speedup vs baseline: 1.1990x; 1.1990x over previous
"""Trainium2 Bass kernel for a dense multi-head attention layer (v5).

Problem shapes (hardcoded): B=2, S=4096, H=512, NH=8, HD=64.
Sharding: 16 (batch, head) pairs -> 2 heads per core across 8 cores.

v5 (~275-278us) vs v4: phase A rebalanced (V-copies on ScalarE since
VectorE was overcommitted there; s2p bufs=3 with 3-ktile groups); the
DVE exp takes the FIRST tile of each phase-B group (its scores finish
~230ns earlier, and the 1.4us DVE op then fits the sc-slot reuse
window where the 1.33us SE exp fits the later slot); a new chunk's
first PV defers one group while the previous chunk's evac chain
drains; oproj output copies alternate SE/VE.

v4 changes vs v2 (301.3us -> ~281.5us):
- K-tile groups of 2 batch the PE array-tiling mode switches (row-tiled
  64x128 concurrent scores pairs vs full-array 128x65 PV): each switch
  drains the PE (~100ns), so batching halves that cost.
- O-projection interleaved into the main loop (was a 34us tail running
  at half clock after the HAM saw >3us of PE idle), fused to ONE
  matmul per token tile: ctx rows are pre-scaled by 1/softmax-sum at
  evacuation (SE row copy -> VE reciprocal_approx_fast -> GpSimd
  partition_broadcast -> VE scalar_tensor_tensor), so both heads'
  64-dim blocks concatenate into one K=128 contraction with Wo.
- Per-chunk ctxT tiles avoid false whole-tile deps (evac writes vs
  oproj reads of other chunks).
- Chunk-0 PV is deferred to phase B (pb pool holds the whole chunk) so
  phase A PSUM fits pj+s2p and phase B gets cx triple-buffering, which
  removes chunk-boundary PE stalls (HAM downclock triggers).
- Softmax exp split ScalarE (ACT Exp) / VectorE (custom DVE op
  EXP2R_ANT, bf16 bits via int16, scores pre-scaled on host).

HW facts this leans on (measured): VE ops mis-read PSUM rows at
partition offsets (hop through SE copy first); GpSimd cannot touch
PSUM and its bulk elementwise is ~20x slower than VE (only
partition_broadcast is cheap); DMA cannot read PSUM.
"""

import numpy as np

B, S, H, NH, HD = 2, 4096, 512, 8, 64
N_CORES = 8

_CACHE = {}

# ---- custom DVE op: 2^(Y/128) -> bf16 bits via int16 ----------------------
EXP2_NAME = "EXP2R_ANT"
KM_VAL = float(3 * 2**29)          # magic: round to multiple of 128
EXP2_A = -0.0026979539543390274    # quadratic coeff (F^2)
EXP2_B = -0.0020175932440906763      # quadratic coeff (F)
EXP2_CE = 16253.2333984375           # 128*127 + fit const (+trunc offset)
LN2_128 = float(np.log(2.0) / 128.0)
SCALE_Q = float(128.0 * np.log2(np.e) / np.sqrt(HD))


def _exp2_reference(in0, in1, s0, s1, imm2):
    f32 = np.float32
    KM = f32(KM_VAL)
    Y = in0.astype(f32)
    G = ((Y + KM).astype(f32) - KM).astype(f32)
    F = (Y - G).astype(f32)
    return ((Y + f32(s0)).astype(f32)
            + (((F * f32(s1)).astype(f32) + f32(imm2)).astype(f32)
               * F).astype(f32)).astype(f32)


def _get_exp2_op():
    import concourse.dve_ops as dvo
    for op in dvo.OPS:
        if op.name == EXP2_NAME:
            return op
    from concourse.dve_spec import (
        C0, C1, C2, C3, Spec, Src0, _spill_c3_to_src1, lower,
    )
    from concourse.dve_uop import DveOpSpec

    Y = Src0
    G = (Y + C3) - C3
    F = Y - G
    body = (Y + C0) + ((F * C1 + C2) * F)
    body = _spill_c3_to_src1(body)
    spec = Spec(body=body, reference=_exp2_reference)

    row = dvo._CUSTOM_DVE_ROW_BASE + len(dvo.OPS)
    shas = {}
    for ver in ("v3", "v4"):
        try:
            uops = lower(spec, ver=ver)
            shas[ver] = DveOpSpec(
                name=EXP2_NAME, opcode=row, uops=uops, rd1_en=True
            ).sha(ver)
        except Exception:
            pass
    assert shas, "EXP2R_ANT failed to lower"
    op = dvo.DveOp(EXP2_NAME, spec, subdim=False, uops_sha=shas)
    dvo.OPS.append(op)
    dvo._SUB_OPCODE_FOR_NAME[EXP2_NAME] = row
    dvo.CUSTOM_DVE_SPECS[EXP2_NAME] = spec
    return op


def _emit(nc, tc, ctx, aps, S_):
    from collections import deque

    import concourse.mybir as mybir

    f32 = mybir.dt.float32
    bf16 = mybir.dt.bfloat16
    i16 = mybir.dt.int16
    Exp = mybir.ActivationFunctionType.Exp
    mult = mybir.AluOpType.mult

    exp2_op = _get_exp2_op()

    NQ = S_ // 512   # query chunks
    NK = S_ // 128   # key tiles
    NS = S_ // 128   # output row tiles

    P = ctx.enter_context(tc.tile_pool(name="persist", bufs=1))

    wk_sb = P.tile([128, 512], bf16, tag="wk")
    nc.sync.dma_start(wk_sb[:], aps["wk"])
    bk_sb = P.tile([128, 1], f32, tag="bk")
    wq_sb = P.tile([128, 512], bf16, tag="wq")
    bq_sb = P.tile([128, 1], f32, tag="bq")
    wv_sb = P.tile([128, 512], bf16, tag="wv")
    wo_sb = P.tile([128, 512], bf16, tag="wo")

    qT = P.tile([128, S_], bf16, tag="qT")         # q^T, 2 heads on partitions
    kT = P.tile([128, S_], bf16, tag="kT")
    Vt = P.tile([128, NK * 130], bf16, tag="V")    # per k-tile: [h0|1s|h1|1s]
    # scaled ctx, one tile per chunk (separate tiles avoid false whole-tile
    # deps between each chunk's evac write and other chunks' oproj reads)
    ctxTs = [P.tile([128, 512], bf16, tag=f"ctxT{c}", name=f"ctxT{c}")
             for c in range(NQ)]
    km = P.tile([128, 1], f32, tag="km")
    nc.vector.memset(km[:], KM_VAL)
    ones1 = P.tile([128, 1], f32, tag="ones1")
    nc.vector.memset(ones1[:], 1.0)
    vt_cols = Vt.rearrange("p (n c) -> p n c", c=65)
    nc.vector.tensor_copy(
        vt_cols[:, :, 64:65], ones1[:, 0:1].to_broadcast((128, 2 * NK, 1))
    )

    nmp = ctx.enter_context(tc.tile_pool(name="nm", bufs=2))
    # Holds all of chunk 0's probs (PV for chunk 0 is deferred to phase B so
    # phase A PSUM fits in pj+s2p) plus the steady-state pipeline depth.
    pbp = ctx.enter_context(tc.tile_pool(name="pb", bufs=34))

    def scores_mm(sc_slice, h, t, c):
        nc.tensor.matmul(
            sc_slice,
            kT[h * 64:(h + 1) * 64, t * 128:(t + 1) * 128],
            qT[h * 64:(h + 1) * 64, c * 512:(c + 1) * 512],
            start=True, stop=True,
        )

    def pv_mm(cxt, h, t, pb_slice):
        nc.tensor.matmul(
            cxt,
            Vt[:, t * 130 + h * 65:t * 130 + (h + 1) * 65],
            pb_slice,
            start=(t == 0), stop=(t == NK - 1),
        )

    def do_exp(sc, pb, use_dve):
        # sc: [128, 1024] f32 PSUM (scores pre-scaled to Y = 128*log2e*s/8)
        # pb: [128, 1024] int16 SBUF; bf16 bit patterns of exp(s/8) land here.
        if use_dve:
            nc.vector._custom_dve(
                exp2_op, out=pb[:], in0=sc[:], in1=km[:, 0:1],
                s0=EXP2_CE, s1=EXP2_A, imm2=EXP2_B,
            )
        else:
            nc.scalar.activation(pb.bitcast(bf16), sc[:], Exp, scale=LN2_128)

    # ---- evacuation: reciprocal of sums row, broadcast, scale ctx --------
    rec_bs = {}

    def make_steps(cx, c2):
        # Paced steps per chunk. s2a (ScalarE) evacuates the cx PSUM slot as
        # soon as possible; the 1/sums chain (SE copy -> VE recip -> gpsimd
        # bcast) runs in parallel; the final multiply runs on idle GpSimd so
        # no step waits behind the VE/SE exp backlog.
        def s1(h):
            def f():
                # VE ops mis-read PSUM rows at partition offsets on HW; hop
                # the sums row to SBUF partition 0 via ScalarE first.
                srow = nmp.tile([1, 512], f32, tag=f"srow{h}", name=f"srow{h}")
                nc.scalar.copy(srow[:], cx[h][64:65, :])
                rec = nmp.tile([1, 512], f32, tag=f"rec{h}", name=f"rec{h}")
                nc.vector.reciprocal_approx_fast(rec[:], srow[:])
                rb = nmp.tile([64, 512], f32, tag=f"recb{h}", name=f"recb{h}")
                nc.gpsimd.partition_broadcast(rb[:], rec[0:1, :])
                rec_bs[(c2, h)] = rb
            return f

        def s2(h):
            def f():
                rb = rec_bs.pop((c2, h))
                nc.vector.scalar_tensor_tensor(
                    ctxTs[c2][h * 64:(h + 1) * 64, :],
                    cx[h][0:64, :], 1.0, rb[:],
                    op0=mult, op1=mult,
                )
            return f
        return [s1(0), s1(1), s2(0), s2(1)]

    obp = ctx.enter_context(tc.tile_pool(name="ob", bufs=3))

    def oproj_tile(st, opp):
        ps = opp.tile([128, 512], f32, tag="op", name="ps")
        nc.tensor.matmul(
            ps[:],
            ctxTs[st // 4][:, (st % 4) * 128:(st % 4 + 1) * 128],
            wo_sb[:],
            start=True, stop=True,
        )
        ob = obp.tile([128, 512], bf16, tag="ob", name="ob")
        if st % 2 == 0:
            nc.scalar.copy(ob[:], ps[:])
        else:
            nc.vector.tensor_copy(ob[:], ps[:])
        nc.sync.dma_start(aps["outp"][st * 128:(st + 1) * 128, :], ob[:])

    # ---- Phase A: projections interleaved with chunk-0 attention ---------
    pend = deque()
    steps = deque()  # ordered (kind, fn): evac s1/s2 then that chunk's oproj
    cxs = {}
    opp_holder = []

    def pop_steps(n):
        popped_op = False
        for _ in range(n):
            if not steps:
                break
            kind, fn = steps[0]
            if kind == "op" and popped_op:
                break
            steps.popleft()
            popped_op |= kind == "op"
            if fn is not None:
                fn()

    with tc.tile_pool(name="xt", bufs=1) as xtp, \
         tc.tile_pool(name="pj", bufs=2, space="PSUM") as pj, \
         tc.tile_pool(name="s2", bufs=3, space="PSUM") as s2p:
        xts = []
        for j in range(4):
            t = xtp.tile([128, S_], bf16, tag=f"xt{j}")
            xts.append(t)
        for j in range(4):
            nc.sync.dma_start(
                xts[j][:, 0:512], aps["xT"][j * 128:(j + 1) * 128, 0:512]
            )
        nc.sync.dma_start(wq_sb[:], aps["wq"])
        nc.sync.dma_start(bk_sb[:], aps["bk"])
        nc.sync.dma_start(bq_sb[:], aps["bq"])
        nc.sync.dma_start(wv_sb[:], aps["wv"])
        for c in range(1, NQ):
            for j in range(4):
                nc.sync.dma_start(
                    xts[j][:, c * 512:(c + 1) * 512],
                    aps["xT"][j * 128:(j + 1) * 128, c * 512:(c + 1) * 512],
                )
        nc.sync.dma_start(wo_sb[:], aps["wo"])

        def kq_proj_chunk(wsb, bsb, dst, c):
            ps = pj.tile([128, 512], f32, tag="pj", name="pjt")
            for j in range(4):
                nc.tensor.matmul(
                    ps[:],
                    wsb[:, j * 128:(j + 1) * 128],
                    xts[j][:, c * 512:(c + 1) * 512],
                    start=(j == 0), stop=(j == 3),
                )
            nc.vector.tensor_scalar_add(
                dst[:, c * 512:(c + 1) * 512], ps[:], bsb[:, 0:1]
            )

        def v_proj_tile(t):
            ps = pj.tile([128, 128], f32, tag="pj", name="pjv")
            for j in range(4):
                nc.tensor.matmul(
                    ps[:],
                    xts[j][:, t * 128:(t + 1) * 128],
                    wv_sb[:, j * 128:(j + 1) * 128],
                    start=(j == 0), stop=(j == 3),
                )
            # ScalarE: phase A's VectorE is the overcommitted engine (exps +
            # bias adds); SE has slack here.
            dst = Vt[:, t * 130:t * 130 + 130].rearrange(
                "p (two c) -> p two c", c=65)[:, :, 0:64]
            src = ps.rearrange("p (two c) -> p two c", c=64)
            nc.scalar.copy(dst, src)

        kq_proj_chunk(wk_sb, bk_sb, kT, 0)
        kq_proj_chunk(wq_sb, bq_sb, qT, 0)
        NVUP = min(1, NK)
        for t in range(NVUP):
            v_proj_tile(t)

        # Interleave: 3 V tiles then one K chunk, repeating, so V stays just
        # ahead of chunk-0 PV consumption and K chunk c lands before tile 4c.
        tasks = []
        vq = list(range(NVUP, NK))
        kq = list(range(1, NQ))
        while vq or kq:
            for _ in range(3):
                if vq:
                    tasks.append(("v", vq.pop(0)))
            if kq:
                tasks.append(("k", kq.pop(0)))
        tasks += [("q", c) for c in range(1, NQ)]
        ti = 0

        GA = 3  # phase A group size (= s2p bufs; one group of exp slack)
        for gstart in range(0, NK, GA):
            for t in range(gstart, min(gstart + GA, NK)):
                sc = s2p.tile([128, 1024], f32, tag="s2", name="s2")
                scores_mm(sc[:, 0:512], 0, t, 0)
                scores_mm(sc[:, 512:1024], 1, t, 0)
                pb = pbp.tile([128, 1024], i16, tag="pb", name="pb")
                do_exp(sc, pb, use_dve=(t % 2 == 0))
                pend.append((0, t, pb.bitcast(bf16)))
            # full-array phase: projections + delayed PV
            t_hi = min(gstart + GA, NK) - 1
            while ti * NK < (t_hi + 1) * len(tasks):
                kind, v = tasks[ti]
                ti += 1
                if kind == "k":
                    kq_proj_chunk(wk_sb, bk_sb, kT, v)
                elif kind == "q":
                    kq_proj_chunk(wq_sb, bq_sb, qT, v)
                else:
                    v_proj_tile(v)

    # ---- Phase B: chunks 1..NQ-1, grouped mode-batched pipeline ----------
    with tc.tile_pool(name="sc", bufs=2, space="PSUM") as scp, \
         tc.tile_pool(name="op", bufs=1, space="PSUM") as opp, \
         tc.tile_pool(name="cx", bufs=3, space="PSUM") as cxp:
        opp_holder.append(opp)
        deferred = set()

        def flush_one():
            c2, t2, pbb = pend.popleft()
            if t2 == 0:
                cxs[c2] = {h: cxp.tile([65, 512], f32, tag="cx",
                                       name=f"cx{h}") for h in (0, 1)}
            cx = cxs[c2]
            pv_mm(cx[0][:], 0, t2, pbb[:, 0:512])
            pv_mm(cx[1][:], 1, t2, pbb[:, 512:1024])
            if t2 == NK - 1:
                for s in make_steps(cx, c2):
                    steps.append(("e", s))
                # pacing slots: give the VE evac chain ~2 groups of headroom
                # before the first oproj matmul enters the PE FIFO
                steps.append(("e", None))
                steps.append(("e", None))
                for j in range(4):
                    steps.append(
                        ("op",
                         lambda st=4 * c2 + j: oproj_tile(st, opp_holder[0])))
                del cxs[c2]

        for c in range(1, NQ):
            for g in range(NK // 2):
                # scores first (tiled mode, back-to-back pairs)
                scg = []
                for t in (2 * g, 2 * g + 1):
                    sc = scp.tile([128, 1024], f32, tag="sc", name="sc")
                    scores_mm(sc[:, 0:512], 0, t, c)
                    scores_mm(sc[:, 512:1024], 1, t, c)
                    scg.append((t, sc))
                # paced evac/oproj steps: their SE/VE ops enqueue ahead of
                # this group's exps so the chain isn't stuck behind them.
                pop_steps(3)
                for t, sc in scg:
                    pb = pbp.tile([128, 1024], i16, tag="pb", name="pb")
                    do_exp(sc, pb, use_dve=(t % 2 == 0))
                    pend.append((c, t, pb.bitcast(bf16)))
                thr = 6
                nfl = 0
                while len(pend) > thr and nfl < 3:
                    # defer a new chunk's first PV by one group while the
                    # previous chunk's evac chain is still draining: ~0.9us
                    # of PE idle now beats ~1.4us of cx-slot wait next group
                    if pend[0][1] == 0 and steps and c not in deferred:
                        deferred.add(c)
                        # keep the PE "busy" through the deferral bubble so
                        # the HAM clock-gate doesn't halve the clock: dummy
                        # weight loads (every real matmul reloads its own
                        # weights, so the background buffer is scratch)
                        for _ in range(8):
                            nc.tensor.ldweights(wk_sb[:, 0:128])
                        break
                    flush_one()
                    nfl += 1
        while pend:
            flush_one()
        # drain the last chunk's evacuation inside these pools; its oproj
        # tiles run after, in a wider PSUM pool so they overlap
        while steps and steps[0][0] == "e":
            _, fn = steps.popleft()
            if fn is not None:
                fn()

    with tc.tile_pool(name="op4", bufs=4, space="PSUM") as op4:
        opp_holder[0] = op4
        while steps:
            _, fn = steps.popleft()
            if fn is not None:
                fn()


def _build(S_=S):
    from contextlib import ExitStack

    import concourse.mybir as mybir
    import concourse.tile as tile
    from concourse import bacc

    f32 = mybir.dt.float32
    bf16 = mybir.dt.bfloat16
    nc = bacc.Bacc("TRN2", target_bir_lowering=False, debug=False,
                   num_devices=N_CORES)
    aps = {
        "xT": nc.dram_tensor("xT", [H, S_], bf16, kind="ExternalInput").ap(),
        "wq": nc.dram_tensor("wq", [128, H], bf16, kind="ExternalInput").ap(),
        "wk": nc.dram_tensor("wk", [128, H], bf16, kind="ExternalInput").ap(),
        "wv": nc.dram_tensor("wv", [128, H], bf16, kind="ExternalInput").ap(),
        "wo": nc.dram_tensor("wo", [128, H], bf16, kind="ExternalInput").ap(),
        "bq": nc.dram_tensor("bq", [128, 1], f32, kind="ExternalInput").ap(),
        "bk": nc.dram_tensor("bk", [128, 1], f32, kind="ExternalInput").ap(),
        "outp": nc.dram_tensor("outp", [S_, H], bf16, kind="ExternalOutput").ap(),
    }
    with tile.TileContext(nc) as tc:
        with ExitStack() as stack:
            _emit(nc, tc, stack, aps, S_)
    nc.compile()
    return nc


def _rearrange_w(wT_slice):
    # [512, 128] (h_in, d) -> [128, 512] where col t*128+d holds wT[t*128+p, d]
    import ml_dtypes

    return np.ascontiguousarray(
        wT_slice.reshape(4, 128, 128).transpose(1, 0, 2).reshape(128, 512)
    ).astype(ml_dtypes.bfloat16)


def _host_prep(hidden_states, Wq, bq, Wk, bk, Wv, bv, Wo, bo, S_=S):
    import ml_dtypes

    bf = ml_dtypes.bfloat16
    xT = [np.ascontiguousarray(hidden_states[b].T).astype(bf)
          for b in range(B)]
    in_maps = []
    for c in range(N_CORES):
        b, p = c // 4, c % 4
        sl = slice(p * 128, (p + 1) * 128)
        in_maps.append({
            "xT": xT[b],
            "wq": _rearrange_w(Wq.T[:, sl].astype(np.float32) * SCALE_Q),
            "wk": _rearrange_w(Wk.T[:, sl].astype(np.float32)),
            "wv": _rearrange_w(Wv.T[:, sl].astype(np.float32)),
            "wo": np.ascontiguousarray(
                Wo.T[p * 128:(p + 1) * 128, :]).astype(bf),
            "bq": np.ascontiguousarray(
                (bq[sl].astype(np.float32) * SCALE_Q).reshape(128, 1)),
            "bk": np.ascontiguousarray(
                bk[sl].astype(np.float32).reshape(128, 1)),
        })
    return in_maps


def kernel(hidden_states, Wq, bq, Wk, bk, Wv, bv, Wo, bo):
    from concourse.bass_utils import run_bass_kernel_spmd

    hidden_states = np.asarray(hidden_states)
    Wq, bq = np.asarray(Wq), np.asarray(bq)
    Wk, bk = np.asarray(Wk), np.asarray(bk)
    Wv, bv = np.asarray(Wv), np.asarray(bv)
    Wo, bo = np.asarray(Wo), np.asarray(bo)

    if "nc" not in _CACHE:
        _CACHE["nc"] = _build(S)
    nc = _CACHE["nc"]

    in_maps = _host_prep(hidden_states, Wq, bq, Wk, bk, Wv, bv, Wo, bo)
    res = run_bass_kernel_spmd(nc, in_maps, core_ids=list(range(N_CORES)))

    # bv contributes bv @ Wo.T (softmax weights sum to 1); bo added once.
    bo_eff = (bo.astype(np.float64) +
              bv.astype(np.float64) @ Wo.T.astype(np.float64)).astype(np.float32)
    out = np.empty((B, S, H), dtype=np.float32)
    for b in range(B):
        acc = np.zeros((S, H), dtype=np.float32)
        for p in range(4):
            acc += res.results[b * 4 + p]["outp"].astype(np.float32)
        out[b] = acc + bo_eff
    return out
